# revision 1
# baseline (speedup 1.0000x reference)
"""Trainium2 Bass kernel for a 2-layer LSTM classifier.

Model (see original nn.Module):
  x  = embedding[features]            # [B, T, E]
  h1 = LSTM_1(x)      (E=8   -> H=256, TF gate order i,j,f,o, forget bias 1.0)
  h2 = LSTM_2(h1)     (H=256 -> H=256)
  out = h2[:, -1] @ Wd + bd           # [B, V]

B=2048, T=80, V=80, E=8, H=256.

Strategy (data-parallel over batch, 8 cores x 256 rows):
  * Everything on-chip lives TRANSPOSED: state h/c are [H, B_local] so the
    per-step matmuls keep the (tiny, shared) weights as the PE stationary
    operand and stream batch columns. Matmul operands are bf16 (1 cyc/row on
    the PE vs 4 for fp32); PSUM accumulation and gate math stay fp32.
  * Layer-1 input path: emb_proj = embedding @ W1[:E] + b1 (+forget bias on
    f columns) is folded on host into an [V, 4H] table; per step the device
    computes z1_x via a one-hot matmul (onehot built on host, streamed),
    which lands directly in the same PSUM accumulation as the h-matmul.
  * Gate columns of all weights are permuted on host to [f | i | o | j].
    Per layer the ACT work is: sigmoid(f,i) [128,1024], tanh(j) [128,512],
    sigmoid(o) [128,512], tanh(c) [128,512] — split so the DVE c-chain can
    start as soon as f,i are known while o's sigmoid runs in parallel.
  * Biases: b1 via emb_proj; b2 (+layer-2 forget bias) via K=1 bias-row
    matmuls accumulated straight into the z2 PSUM; bd via ACT bias on the
    final copy.
  * PSUM has_written semantics: start=True clears the WHOLE bank's bits, so
    each bank gets exactly one start=True MM (its first write); the bank
    neighbour's first write relies on has_written=0 = overwrite.
"""

import os
import sys

import ml_dtypes
import numpy as np

BF16 = ml_dtypes.bfloat16

for _p in ("/root/.axon_site/_ro/trn_rl_repo", "/opt/trn_rl_repo"):
    if os.path.isdir(_p) and _p not in sys.path:
        sys.path.insert(0, _p)

B, T, V, E, H = 2048, 80, 80, 8, 256
FB = 1.0  # forget-gate bias
NCORES = 8
BL = B // NCORES  # 256 batch rows per core
G4 = 4 * H  # 1024
NM = G4 // 128  # 8 output chunks of 128

# gate order in reference W columns: i=[0:256] j=[256:512] f=[512:768] o=[768:1024]
# on-chip order: [f | i | o | j]
_PERM = None

# bank emission order: f, i banks first (unblock sigmoid(f,i)), then j
# (unblock tanh(j)), then o
BANK_ORDER = (0, 1, 3, 2)


def _perm():
    global _PERM
    if _PERM is None:
        ar = np.arange
        _PERM = np.concatenate(
            [ar(512, 768), ar(0, 256), ar(768, 1024), ar(256, 512)]
        )
    return _PERM


_CACHE = {}


def _build_nc(fb_chunks, n_steps=T, debug_dump=False):
    """Build the (SPMD, per-core) bass program.

    fb_chunks: tuple of 128-col chunk indices whose layer-2 bias row is
    nonzero; each gets a K=1 bias-row matmul accumulated into z2.
    """
    import concourse.tile as tile
    from concourse import bacc, mybir

    f32 = mybir.dt.float32
    bf16 = mybir.dt.bfloat16
    AF = mybir.ActivationFunctionType

    nc = bacc.Bacc("TRN2", target_bir_lowering=False, debug=False)

    onehot_d = nc.dram_tensor("onehot", [T, V, BL], bf16, kind="ExternalInput")
    w1h_d = nc.dram_tensor("w1h", [2, 128, G4], bf16, kind="ExternalInput")
    w2x_d = nc.dram_tensor("w2x", [2, 128, G4], bf16, kind="ExternalInput")
    w2h_d = nc.dram_tensor("w2h", [2, 128, G4], bf16, kind="ExternalInput")
    embp_d = nc.dram_tensor("embp", [V, G4], bf16, kind="ExternalInput")
    wd_d = nc.dram_tensor("wd", [2, 128, V], bf16, kind="ExternalInput")
    bdt_d = nc.dram_tensor("bdt", [V, 1], f32, kind="ExternalInput")
    # layer-2 bias row (b2 + forget bias), permuted gate order
    brow_d = nc.dram_tensor("brow", [1, G4], bf16, kind="ExternalInput")
    out_d = nc.dram_tensor("out", [V, BL], f32, kind="ExternalOutput")
    if debug_dump:
        dbg_d = {
            name: nc.dram_tensor(f"dbg_{name}", [128, shp], f32,
                                 kind="ExternalOutput")
            for name, shp in [("c1", 512), ("h1", 512),
                              ("c2", 512), ("h2", 512)]
        }

    with tile.TileContext(nc) as tc:
        with (
            tc.tile_pool(name="wpool", bufs=1) as wpool,
            tc.tile_pool(name="state", bufs=2) as state,
            tc.tile_pool(name="work", bufs=2) as work,
            tc.tile_pool(name="ohpool", bufs=6) as ohpool,
            tc.tile_pool(name="psum", bufs=1, space="PSUM") as psum,
        ):
            # ---- resident weights ----
            w1h = [wpool.tile([128, G4], bf16, tag=f"w1h{k}", name=f"w1h{k}") for k in range(2)]
            w2x = [wpool.tile([128, G4], bf16, tag=f"w2x{k}", name=f"w2x{k}") for k in range(2)]
            w2h = [wpool.tile([128, G4], bf16, tag=f"w2h{k}", name=f"w2h{k}") for k in range(2)]
            embp = wpool.tile([V, G4], bf16, tag="embp", name="embp")
            wd = [wpool.tile([128, V], bf16, tag=f"wd{k}", name=f"wd{k}") for k in range(2)]
            bdt = wpool.tile([V, 1], f32, tag="bdt", name="bdt")
            brow = wpool.tile([1, G4], bf16, tag="brow", name="brow")
            ones1 = wpool.tile([1, BL], bf16, tag="ones1", name="ones1")
            for k in range(2):
                nc.sync.dma_start(out=w1h[k][:], in_=w1h_d[k])
                nc.sync.dma_start(out=w2x[k][:], in_=w2x_d[k])
                nc.sync.dma_start(out=w2h[k][:], in_=w2h_d[k])
                nc.sync.dma_start(out=wd[k][:], in_=wd_d[k])
            nc.sync.dma_start(out=embp[:], in_=embp_d[:])
            nc.sync.dma_start(out=bdt[:], in_=bdt_d[:])
            nc.sync.dma_start(out=brow[:], in_=brow_d[:])
            nc.gpsimd.memset(ones1[:], 1.0)

            h1 = c1 = h2 = c2 = None
            CH = [slice(0, 256), slice(256, 512)]  # rhs column slices per k-tile

            def cell2_finish(pend):
                """Layer-2 cell of the previous step (skewed pipeline)."""
                nonlocal c2, h2
                sfi2p, tj2p, so2p = pend
                is2 = sfi2p[:, 512:1024]
                c2n = state.tile([128, 512], f32, tag="c2", name="c2")
                if c2 is None:
                    nc.vector.tensor_mul(c2n[:], is2, tj2p[:])
                else:
                    ca2 = work.tile([128, 512], f32, tag="ca2", name="ca2")
                    t12 = work.tile([128, 512], f32, tag="t12", name="t12")
                    nc.vector.tensor_mul(ca2[:], c2[:], sfi2p[:, 0:512])
                    nc.vector.tensor_mul(t12[:], is2, tj2p[:])
                    nc.vector.tensor_add(c2n[:], ca2[:], t12[:])
                thc2 = work.tile([128, 512], f32, tag="thc2", name="thc2")
                nc.scalar.activation(thc2[:], c2n[:], AF.Tanh)
                h2n = state.tile([128, 512], bf16, tag="h2", name="h2")
                nc.vector.tensor_mul(h2n[:], thc2[:], so2p[:])
                c2, h2 = c2n, h2n

            pend = None  # layer-2 gate tiles of step t-1 awaiting the cell
            for t in range(n_steps):
                oh = ohpool.tile([V, BL], bf16, tag="oh", name=f"oh{t}")
                nc.sync.dma_start(out=oh[:], in_=onehot_d[t])

                # ---------- layer 1: z1 = W1h.T @ h1 + embp.T @ onehot ------
                z1 = psum.tile([128, 2048], f32, tag="z1", name=f"z1_{t}")
                for bk in BANK_ORDER:
                    m0, m1 = 2 * bk, 2 * bk + 1
                    sl = {m: z1[:, 256 * m : 256 * (m + 1)] for m in (m0, m1)}
                    wsl = {m: slice(128 * m, 128 * (m + 1)) for m in (m0, m1)}
                    nc.tensor.matmul(sl[m0], embp[:, wsl[m0]], oh[:],
                                     start=True, stop=False)
                    nc.tensor.matmul(sl[m1], embp[:, wsl[m1]], oh[:],
                                     start=False, stop=(h1 is None))
                    if h1 is not None:
                        for m in (m0, m1):
                            nc.tensor.matmul(sl[m], w1h[0][:, wsl[m]], h1[:, CH[0]],
                                             start=False, stop=False)
                            nc.tensor.matmul(sl[m], w1h[1][:, wsl[m]], h1[:, CH[1]],
                                             start=False, stop=(m == m1))

                # ---------- layer-1 f/i sigmoid, then the SKEWED layer-2
                # cell of step t-1 (its inputs are long ready, so its DVE/ACT
                # work fills the queues without delaying sigmoid(f,i)) ------
                sfi1 = work.tile([128, 1024], f32, tag="sfi1", name="sfi1")
                nc.scalar.activation(sfi1[:], z1[:, 0:1024], AF.Sigmoid)
                if c1 is not None:
                    ca1 = work.tile([128, 512], f32, tag="ca1", name="ca1")
                    nc.vector.tensor_mul(ca1[:], c1[:], sfi1[:, 0:512])
                if pend is not None:
                    cell2_finish(pend)  # produces h2[t-1] (thc2 + h2 mul)
                    pend = None

                # ------- layer 2 part A (h2-recurrent + bias rows) -------
                z2 = psum.tile([128, 2048], f32, tag="z2", name=f"z2_{t}")
                if h2 is not None:
                    for bk in BANK_ORDER:
                        m0, m1 = 2 * bk, 2 * bk + 1
                        for i, (m, k) in enumerate(
                            [(m0, 0), (m1, 0), (m0, 1), (m1, 1)]
                        ):
                            nc.tensor.matmul(z2[:, 256 * m : 256 * (m + 1)],
                                             w2h[k][:, 128 * m : 128 * (m + 1)],
                                             h2[:, CH[k]],
                                             start=(i == 0), stop=False)
                    for m in fb_chunks:
                        nc.tensor.matmul(z2[:, 256 * m : 256 * (m + 1)],
                                         brow[:, 128 * m : 128 * (m + 1)],
                                         ones1[:], start=False, stop=False)

                # ---------- rest of the layer-1 gates + cell ----------
                tj1 = work.tile([128, 512], f32, tag="tj1", name="tj1")
                nc.scalar.activation(tj1[:], z1[:, 1536:2048], AF.Tanh)
                t11 = work.tile([128, 512], f32, tag="t11", name="t11")
                nc.vector.tensor_mul(t11[:], sfi1[:, 512:1024], tj1[:])
                so1 = work.tile([128, 512], f32, tag="so1", name="so1")
                nc.scalar.activation(so1[:], z1[:, 1024:1536], AF.Sigmoid)
                c1n = state.tile([128, 512], f32, tag="c1", name="c1")
                if c1 is None:
                    nc.vector.tensor_copy(c1n[:], t11[:])
                else:
                    nc.vector.tensor_add(c1n[:], ca1[:], t11[:])
                thc1 = work.tile([128, 512], f32, tag="thc1", name="thc1")
                nc.scalar.activation(thc1[:], c1n[:], AF.Tanh)
                h1n = state.tile([128, 512], bf16, tag="h1", name="h1")
                nc.vector.tensor_mul(h1n[:], thc1[:], so1[:])
                c1, h1 = c1n, h1n

                # ---------- layer 2 part B (h1 input) ----------
                first2 = h2 is None
                for bk in BANK_ORDER:
                    m0, m1 = 2 * bk, 2 * bk + 1
                    mms = [(m, w2x[k][:, 128 * m : 128 * (m + 1)], h1[:, CH[k]])
                           for m, k in [(m0, 0), (m1, 0), (m0, 1), (m1, 1)]]
                    if first2:  # bias rows ride in this group at t=0
                        mms += [(m, brow[:, 128 * m : 128 * (m + 1)], ones1[:])
                                for m in (m0, m1) if m in fb_chunks]
                    for i, (m, lhsT, rhs) in enumerate(mms):
                        nc.tensor.matmul(z2[:, 256 * m : 256 * (m + 1)], lhsT, rhs,
                                         start=(first2 and i == 0),
                                         stop=(i == len(mms) - 1))

                # ---------- layer 2 gates (cell is finished next step) ------
                sfi2 = work.tile([128, 1024], f32, tag="sfi2", name="sfi2")
                nc.scalar.activation(sfi2[:], z2[:, 0:1024], AF.Sigmoid)
                tj2 = work.tile([128, 512], f32, tag="tj2", name="tj2")
                nc.scalar.activation(tj2[:], z2[:, 1536:2048], AF.Tanh)
                so2 = work.tile([128, 512], f32, tag="so2", name="so2")
                nc.scalar.activation(so2[:], z2[:, 1024:1536], AF.Sigmoid)
                pend = (sfi2, tj2, so2)

            cell2_finish(pend)  # epilogue: layer-2 cell of the last step

            if debug_dump:
                for name, tl in [("c1", c1), ("c2", c2)]:
                    nc.sync.dma_start(out=dbg_d[name][:], in_=tl[:])
                for name, tl in [("h1", h1), ("h2", h2)]:
                    hb = work.tile([128, 512], f32, tag=f"d{name}", name=f"d{name}")
                    nc.vector.tensor_copy(hb[:], tl[:])
                    nc.sync.dma_start(out=dbg_d[name][:], in_=hb[:])

            # ---------- dense head on final h2 ----------
            lg = psum.tile([128, 2048], f32, tag="z1", name="lg")
            nc.tensor.matmul(lg[0:V, 0:BL], wd[0][:], h2[:, CH[0]],
                             start=True, stop=False)
            nc.tensor.matmul(lg[0:V, 0:BL], wd[1][:], h2[:, CH[1]],
                             start=False, stop=True)
            outs = work.tile([V, BL], f32, tag="outs", name="outs")
            nc.scalar.add(outs[:], lg[0:V, 0:BL], bdt[:])
            nc.sync.dma_start(out=out_d[:], in_=outs[:])

    nc.compile()
    return nc


def _get_nc(fb_chunks):
    key = ("nc", fb_chunks)
    if key not in _CACHE:
        _CACHE[key] = _build_nc(fb_chunks)
    return _CACHE[key]


def _prep_inputs(features, embedding, W1, b1, W2, b2, Wd, bd):
    """Host-side weight folding / layout prep -> (per-core input maps, fb_chunks)."""
    features = np.asarray(features, np.int32)
    embedding = np.asarray(embedding, np.float32)
    W1 = np.asarray(W1, np.float32)
    b1 = np.asarray(b1, np.float32)
    W2 = np.asarray(W2, np.float32)
    b2 = np.asarray(b2, np.float32)
    Wd = np.asarray(Wd, np.float32)
    bd = np.asarray(bd, np.float32)

    p = _perm()
    W1p = W1[:, p]
    W2p = W2[:, p]
    b1p = b1[p]
    b2p = b2[p]
    fbvec = np.zeros(G4, np.float32)
    fbvec[0:256] = FB  # f block sits first in permuted order

    embp = (embedding @ W1p[:E] + (b1p + fbvec)).astype(BF16)  # [V, 4H]
    w1h = np.ascontiguousarray(W1p[E:].reshape(2, 128, G4).astype(BF16))
    w2x = np.ascontiguousarray(W2p[:H].reshape(2, 128, G4).astype(BF16))
    w2h = np.ascontiguousarray(W2p[H:].reshape(2, 128, G4).astype(BF16))
    wd = np.ascontiguousarray(Wd.reshape(2, 128, V).astype(BF16))
    bdt = np.ascontiguousarray(bd.reshape(V, 1))
    b2full = (b2p + fbvec).astype(np.float32)
    brow = np.ascontiguousarray(b2full.reshape(1, G4).astype(BF16))
    fb_chunks = tuple(
        m for m in range(NM) if np.any(b2full[128 * m : 128 * (m + 1)] != 0.0)
    )

    eye = np.eye(V, dtype=BF16)
    shared = {
        "w1h": w1h, "w2x": w2x, "w2h": w2h, "embp": embp,
        "wd": wd, "bdt": bdt, "brow": brow,
    }
    in_maps = []
    for c in range(NCORES):
        f = features[c * BL : (c + 1) * BL]  # [BL, T]
        oh = eye[f.T]  # [T, BL, V]
        oh = np.ascontiguousarray(oh.transpose(0, 2, 1))  # [T, V, BL]
        m = dict(shared)
        m["onehot"] = oh
        in_maps.append(m)
    return in_maps, fb_chunks


def _run(in_maps, fb_chunks, trace=False):
    from concourse.bass_utils import run_bass_kernel_spmd

    nc = _get_nc(fb_chunks)
    res = run_bass_kernel_spmd(nc, in_maps, list(range(NCORES)), trace=trace)
    logits = np.concatenate([r["out"].T for r in res.results], axis=0)  # [B, V]
    return logits.astype(np.float32), res


def kernel(features, embedding, W1, b1, W2, b2, Wd, bd):
    in_maps, fb_chunks = _prep_inputs(features, embedding, W1, b1, W2, b2, Wd, bd)
    logits, _ = _run(in_maps, fb_chunks, trace=False)
    return logits



# revision 24
# speedup vs baseline: 1.3909x; 1.3909x over previous
"""Trainium2 Bass kernel for a 2-layer LSTM classifier.

Model:
  x  = embedding[features]            # [B, T, E]
  h1 = LSTM_1(x)      (E=8   -> H=256, TF gate order i,j,f,o, forget bias 1.0)
  h2 = LSTM_2(h1)     (H=256 -> H=256)
  out = h2[:, -1] @ Wd + bd           # [B, V]

B=2048, T=80, V=80, E=8, H=256.

Strategy (data-parallel over batch, 8 cores x 256 rows), fully-skewed
software pipeline so the PE never waits on same-iteration results:

  * Gate-major layout: all on-chip state is [H, B_local]; weights are the
    stationary matmul operand, h streams as the moving operand (bf16,
    1 cyc/row).  Gate columns permuted on host to [f | i | o | j].
  * Iteration t of the emission loop computes: z1[t] h-part (h1[t-1] is one
    iteration old), ALL of z2[t-1] (h1[t-1] and h2[t-2] are old), and the
    onehot part of z1[t+1].  Every matmul therefore depends only on data
    from previous iterations -> PE runs back-to-back at full clock.
  * Layer-1 input path: emb_proj = embedding @ W1[:E] + b1 (+forget bias)
    folded on host into a [V, 4H] table; a one-hot matmul per step
    accumulates it into the same PSUM group as the h-part.
  * Layer-2 forget bias: applied via the activation bias operand on a
    separate sigmoid(f) instruction (fast path, when b2+fb is uniform on
    the f block and zero elsewhere); generic fallback uses K=1 bias-row
    matmuls accumulated into z2.
  * ACT work per step is 8 coarse instructions: sig(f,i,o) [1536] + tanh(j)
    + tanh(c) for layer 1; tanh(j2), sig(f2)+bias, sig(i2), sig(o2),
    tanh(c2) for layer 2 (split so the layer-2 cell chain finishes early
    enough for next iteration's z2).
  * Gates and tanh outputs are bf16 in SBUF (DVE 2x mode for the products);
    cell state c stays fp32.
  * PSUM: z1 in banks 0-3, z2 in banks 4-7; start=True only on each bank's
    first matmul, stop=True on its last (has_written semantics).
"""

import os
import sys

import ml_dtypes
import numpy as np

BF16 = ml_dtypes.bfloat16

for _p in ("/root/.axon_site/_ro/trn_rl_repo", "/opt/trn_rl_repo"):
    if os.path.isdir(_p) and _p not in sys.path:
        sys.path.insert(0, _p)

B, T, V, E, H = 2048, 80, 80, 8, 256
FB = 1.0  # forget-gate bias
NCORES = 8
BL = B // NCORES  # 256 batch rows per core
G4 = 4 * H  # 1024
NM = G4 // 128  # 8 output chunks of 128 gate rows

# on-chip gate order: [f | i | o | j]; chunk m covers gate rows 128m..128m+127
# bank b holds chunks (2b, 2b+1):  f=bank0, i=bank1, o=bank2, j=bank3
_PERM = None


def _perm():
    global _PERM
    if _PERM is None:
        ar = np.arange
        _PERM = np.concatenate(
            [ar(512, 768), ar(0, 256), ar(768, 1024), ar(256, 512)]
        )
    return _PERM


_CACHE = {}


def _build_nc(fast_bias, fb_chunks, n_steps=T):
    """Build the (SPMD, per-core) bass program.

    fast_bias: layer-2 bias handled by the sigmoid(f2) activation bias
    operand (b2+fb uniform across the two f chunks, zero elsewhere).
    fb_chunks: chunk indices needing K=1 bias-row matmuls (generic path).
    """
    import concourse.tile as tile
    from concourse import bacc, mybir

    f32 = mybir.dt.float32
    bf16 = mybir.dt.bfloat16
    AF = mybir.ActivationFunctionType

    nc = bacc.Bacc("TRN2", target_bir_lowering=False, debug=False)

    onehot_d = nc.dram_tensor("onehot", [T, V, BL], bf16, kind="ExternalInput")
    w1h_d = nc.dram_tensor("w1h", [2, 128, G4], bf16, kind="ExternalInput")
    w2x_d = nc.dram_tensor("w2x", [2, 128, G4], bf16, kind="ExternalInput")
    w2h_d = nc.dram_tensor("w2h", [2, 128, G4], bf16, kind="ExternalInput")
    embp_d = nc.dram_tensor("embp", [V, G4], bf16, kind="ExternalInput")
    wd_d = nc.dram_tensor("wd", [2, 128, V], bf16, kind="ExternalInput")
    bdt_d = nc.dram_tensor("bdt", [V, 1], f32, kind="ExternalInput")
    brow_d = nc.dram_tensor("brow", [1, G4], bf16, kind="ExternalInput")
    out_d = nc.dram_tensor("out", [V, BL], f32, kind="ExternalOutput")

    with tile.TileContext(nc) as tc:
        with (
            tc.tile_pool(name="wpool", bufs=1) as wpool,
            tc.tile_pool(name="state", bufs=2) as state,
            tc.tile_pool(name="work", bufs=2) as work,
            tc.tile_pool(name="ohpool", bufs=6) as ohpool,
            tc.tile_pool(name="psum", bufs=1, space="PSUM") as psum,
        ):
            # ---- resident weights ----
            w1h = [wpool.tile([128, G4], bf16, tag=f"w1h{k}", name=f"w1h{k}") for k in range(2)]
            w2x = [wpool.tile([128, G4], bf16, tag=f"w2x{k}", name=f"w2x{k}") for k in range(2)]
            w2h = [wpool.tile([128, G4], bf16, tag=f"w2h{k}", name=f"w2h{k}") for k in range(2)]
            embp = wpool.tile([V, G4], bf16, tag="embp", name="embp")
            wd = [wpool.tile([128, V], bf16, tag=f"wd{k}", name=f"wd{k}") for k in range(2)]
            bdt = wpool.tile([V, 1], f32, tag="bdt", name="bdt")
            for k in range(2):
                nc.sync.dma_start(out=w1h[k][:], in_=w1h_d[k])
                nc.sync.dma_start(out=w2x[k][:], in_=w2x_d[k])
                nc.sync.dma_start(out=w2h[k][:], in_=w2h_d[k])
                nc.sync.dma_start(out=wd[k][:], in_=wd_d[k])
            nc.sync.dma_start(out=embp[:], in_=embp_d[:])
            nc.sync.dma_start(out=bdt[:], in_=bdt_d[:])
            brow = wpool.tile([1, G4], bf16, tag="brow", name="brow")
            ones1 = wpool.tile([1, BL], bf16, tag="ones1", name="ones1")
            nc.sync.dma_start(out=brow[:], in_=brow_d[:])
            nc.gpsimd.memset(ones1[:], 1.0)

            CH = [slice(0, 256), slice(256, 512)]  # h column slices per k-tile
            h1 = c1 = h2 = c2 = None  # state of iteration t-1

            def oh_mms(z, oh, close):
                """onehot part of a z1 accumulation; start=True per bank."""
                for bk in range(4):
                    m0, m1 = 2 * bk, 2 * bk + 1
                    nc.tensor.matmul(z[:, 256 * m0: 256 * (m0 + 1)],
                                     embp[:, 128 * m0: 128 * (m0 + 1)], oh[:],
                                     start=True, stop=False)
                    nc.tensor.matmul(z[:, 256 * m1: 256 * (m1 + 1)],
                                     embp[:, 128 * m1: 128 * (m1 + 1)], oh[:],
                                     start=False, stop=close)

            # ---------------- prologue ----------------
            oh_tiles = {}
            oh_tiles[0] = ohpool.tile([V, BL], bf16, tag="oh", name="oh0")
            nc.sync.dma_start(out=oh_tiles[0][:], in_=onehot_d[0])
            z1 = psum.tile([128, 2048], f32, tag="z1", name="z1_0")
            oh_mms(z1, oh_tiles[0], close=True)  # t=0 has no h-part

            for t in range(n_steps):
                # prefetch next onehot
                if t + 1 < n_steps:
                    ohn = ohpool.tile([V, BL], bf16, tag="oh", name=f"oh{t + 1}")
                    nc.sync.dma_start(out=ohn[:], in_=onehot_d[t + 1])
                    oh_tiles[t + 1] = ohn
                oh_tiles.pop(t - 1, None)

                # ---- PE: z1[t] h-part (banks f,i,j,o: j early for tanh) ----
                if h1 is not None:
                    for bk in (0, 1, 3, 2):
                        m0, m1 = 2 * bk, 2 * bk + 1
                        for i, (m, k) in enumerate(
                            [(m0, 0), (m0, 1), (m1, 0), (m1, 1)]
                        ):
                            nc.tensor.matmul(z1[:, 256 * m: 256 * (m + 1)],
                                             w1h[k][:, 128 * m: 128 * (m + 1)],
                                             h1[:, CH[k]],
                                             start=False, stop=(i == 3))

                # ---- ACT: layer-1 sigmoid f,i then tanh j (chain head) ----
                g1 = work.tile([128, 1536], bf16, tag="g1", name="g1")
                nc.scalar.activation(g1[:, 0:1024], z1[:, 0:1024], AF.Sigmoid)
                tj1 = work.tile([128, 512], bf16, tag="tj1", name="tj1")
                nc.scalar.activation(tj1[:], z1[:, 1536:2048], AF.Tanh)
                if c1 is not None:
                    ca1 = work.tile([128, 512], bf16, tag="ca1", name="ca1")
                    nc.vector.tensor_mul(ca1[:], c1[:], g1[:, 0:512])

                # ---- PE: all of z2[t-1]; h1-part first, then h2-part ----
                if t >= 1:
                    z2 = psum.tile([128, 2048], f32, tag="z2", name=f"z2_{t - 1}")
                    first = {m: True for m in range(NM)}
                    last_of = {}
                    nmm = 2 if h2 is None else 4
                    for bk in (3, 0, 1, 2):
                        last_of[2 * bk] = last_of[2 * bk + 1] = nmm
                    for part in range(2 if h2 is not None else 1):
                        w2 = w2x if part == 0 else w2h
                        hs = h1 if part == 0 else h2
                        for bk in (3, 0, 1, 2):
                            m0, m1 = 2 * bk, 2 * bk + 1
                            for m, k in [(m0, 0), (m0, 1), (m1, 0), (m1, 1)]:
                                last_of[m] -= 1
                                is_last = (last_of[m] == 0 and
                                           (fast_bias or m not in fb_chunks))
                                nc.tensor.matmul(
                                    z2[:, 256 * m: 256 * (m + 1)],
                                    w2[k][:, 128 * m: 128 * (m + 1)], hs[:, CH[k]],
                                    start=first[m0] and m == m0 and k == 0
                                    and part == 0,
                                    stop=is_last)
                                if m == m0 and k == 0 and part == 0:
                                    first[m0] = False
                    if not fast_bias:
                        for bk in (3, 0, 1, 2):
                            for m in (2 * bk, 2 * bk + 1):
                                if m in fb_chunks:
                                    nc.tensor.matmul(
                                        z2[:, 256 * m: 256 * (m + 1)],
                                        brow[:, 128 * m: 128 * (m + 1)], ones1[:],
                                        start=False, stop=True)

                # ---- DVE: finish layer-1 cell of t ----
                c1n = state.tile([128, 512], bf16, tag="c1", name="c1")
                if c1 is None:
                    nc.vector.tensor_mul(c1n[:], g1[:, 512:1024], tj1[:])
                else:
                    t11 = work.tile([128, 512], bf16, tag="t11", name="t11")
                    nc.vector.tensor_mul(t11[:], g1[:, 512:1024], tj1[:])
                    nc.vector.tensor_add(c1n[:], ca1[:], t11[:])

                # layer-1 sigmoid(o): off the critical chain, before thc1
                nc.scalar.activation(g1[:, 1024:1536], z1[:, 1024:1536], AF.Sigmoid)

                # ---- ACT: layer-2 gates of t-1: tanh(j2), then one merged
                # sigmoid(f,i,o) (bias rows already accumulated into z2) ----
                if t >= 1:
                    tj2 = work.tile([128, 512], bf16, tag="tj2", name="tj2")
                    nc.scalar.activation(tj2[:], z2[:, 1536:2048], AF.Tanh)

                thc1 = work.tile([128, 512], bf16, tag="thc1", name="thc1")
                nc.scalar.activation(thc1[:], c1n[:], AF.Tanh)

                if t >= 1:
                    g2 = work.tile([128, 1536], bf16, tag="g2", name="g2")
                    nc.scalar.activation(g2[:], z2[:, 0:1536], AF.Sigmoid)
                    if c2 is not None:
                        ca2 = work.tile([128, 512], bf16, tag="ca2", name="ca2")
                        nc.vector.tensor_mul(ca2[:], c2[:], g2[:, 0:512])

                h1n = state.tile([128, 512], bf16, tag="h1", name="h1")
                nc.vector.tensor_mul(h1n[:], thc1[:], g1[:, 1024:1536])

                if t >= 1:
                    c2n = state.tile([128, 512], bf16, tag="c2", name="c2")
                    if c2 is None:
                        nc.vector.tensor_mul(c2n[:], g2[:, 512:1024], tj2[:])
                    else:
                        t12 = work.tile([128, 512], bf16, tag="t12", name="t12")
                        nc.vector.tensor_mul(t12[:], g2[:, 512:1024], tj2[:])
                        nc.vector.tensor_add(c2n[:], ca2[:], t12[:])
                    thc2 = work.tile([128, 512], bf16, tag="thc2", name="thc2")
                    nc.scalar.activation(thc2[:], c2n[:], AF.Tanh)
                    h2n = state.tile([128, 512], bf16, tag="h2", name="h2")
                    nc.vector.tensor_mul(h2n[:], thc2[:], g2[:, 1024:1536])
                    h2, c2 = h2n, c2n

                # ---- PE: onehot part of z1[t+1] ----
                if t + 1 < n_steps:
                    z1 = psum.tile([128, 2048], f32, tag="z1", name=f"z1_{t + 1}")
                    oh_mms(z1, oh_tiles[t + 1], close=False)

                h1, c1 = h1n, c1n

            # ---------------- epilogue: z2[T-1] + layer-2 cell ----------------
            z2 = psum.tile([128, 2048], f32, tag="z2", name=f"z2_{n_steps - 1}")
            for bk in (3, 0, 1, 2):
                m0, m1 = 2 * bk, 2 * bk + 1
                mms = [(m, w2x[k][:, 128 * m: 128 * (m + 1)], h1[:, CH[k]])
                       for m, k in [(m0, 0), (m0, 1), (m1, 0), (m1, 1)]]
                if h2 is not None:
                    mms += [(m, w2h[k][:, 128 * m: 128 * (m + 1)], h2[:, CH[k]])
                            for m, k in [(m0, 0), (m0, 1), (m1, 0), (m1, 1)]]
                if not fast_bias:
                    mms += [(m, brow[:, 128 * m: 128 * (m + 1)], ones1[:])
                            for m in (m0, m1) if m in fb_chunks]
                for i, (m, lhsT, rhs) in enumerate(mms):
                    nc.tensor.matmul(z2[:, 256 * m: 256 * (m + 1)], lhsT, rhs,
                                     start=(i == 0), stop=(i == len(mms) - 1))
            tj2 = work.tile([128, 512], bf16, tag="tj2", name="tj2_e")
            nc.scalar.activation(tj2[:], z2[:, 1536:2048], AF.Tanh)
            g2 = work.tile([128, 1536], bf16, tag="g2", name="g2_e")
            nc.scalar.activation(g2[:], z2[:, 0:1536], AF.Sigmoid)
            ca2 = work.tile([128, 512], bf16, tag="ca2", name="ca2_e")
            nc.vector.tensor_mul(ca2[:], c2[:], g2[:, 0:512])
            t12 = work.tile([128, 512], bf16, tag="t12", name="t12_e")
            nc.vector.tensor_mul(t12[:], g2[:, 512:1024], tj2[:])
            c2n = state.tile([128, 512], bf16, tag="c2", name="c2_e")
            nc.vector.tensor_add(c2n[:], ca2[:], t12[:])
            thc2 = work.tile([128, 512], bf16, tag="thc2", name="thc2_e")
            nc.scalar.activation(thc2[:], c2n[:], AF.Tanh)
            h2n = state.tile([128, 512], bf16, tag="h2", name="h2_e")
            nc.vector.tensor_mul(h2n[:], thc2[:], g2[:, 1024:1536])

            # ---------------- dense head on final h2 ----------------
            lg = psum.tile([128, 2048], f32, tag="z1", name="lg")
            nc.tensor.matmul(lg[0:V, 0:BL], wd[0][:], h2n[:, CH[0]],
                             start=True, stop=False)
            nc.tensor.matmul(lg[0:V, 0:BL], wd[1][:], h2n[:, CH[1]],
                             start=False, stop=True)
            outs = work.tile([V, BL], f32, tag="outs", name="outs")
            nc.scalar.add(outs[:], lg[0:V, 0:BL], bdt[:])
            nc.sync.dma_start(out=out_d[:], in_=outs[:])

    nc.compile()
    return nc


def _get_nc(key):
    fast_bias, fb_chunks = key
    ck = ("nc", key)
    if ck not in _CACHE:
        _CACHE[ck] = _build_nc(fast_bias, fb_chunks)
    return _CACHE[ck]


def _prep_inputs(features, embedding, W1, b1, W2, b2, Wd, bd):
    """Host-side weight folding / layout prep -> (per-core input maps, key)."""
    features = np.asarray(features, np.int32)
    embedding = np.asarray(embedding, np.float32)
    W1 = np.asarray(W1, np.float32)
    b1 = np.asarray(b1, np.float32)
    W2 = np.asarray(W2, np.float32)
    b2 = np.asarray(b2, np.float32)
    Wd = np.asarray(Wd, np.float32)
    bd = np.asarray(bd, np.float32)

    p = _perm()
    W1p = W1[:, p]
    W2p = W2[:, p]
    b1p = b1[p]
    b2p = b2[p]
    fbvec = np.zeros(G4, np.float32)
    fbvec[0:256] = FB  # f block sits first in permuted order

    embp = (embedding @ W1p[:E] + (b1p + fbvec)).astype(BF16)  # [V, 4H]
    w1h = np.ascontiguousarray(W1p[E:].reshape(2, 128, G4).astype(BF16))
    w2x = np.ascontiguousarray(W2p[:H].reshape(2, 128, G4).astype(BF16))
    w2h = np.ascontiguousarray(W2p[H:].reshape(2, 128, G4).astype(BF16))
    wd = np.ascontiguousarray(Wd.reshape(2, 128, V).astype(BF16))
    bdt = np.ascontiguousarray(bd.reshape(V, 1))

    b2full = (b2p + fbvec).astype(np.float32)
    fast_bias = False  # bias rows via K=1 matmuls; enables merged sigmoid(f,i,o)
    fb_chunks = tuple(
        m for m in range(NM)
        if np.any(b2full[128 * m: 128 * (m + 1)] != 0.0)
    )
    shared = {
        "w1h": w1h, "w2x": w2x, "w2h": w2h, "embp": embp,
        "wd": wd, "bdt": bdt,
        "brow": np.ascontiguousarray(b2full.reshape(1, G4).astype(BF16)),
    }

    eye = np.eye(V, dtype=BF16)
    in_maps = []
    for c in range(NCORES):
        f = features[c * BL: (c + 1) * BL]  # [BL, T]
        oh = eye[f.T]  # [T, BL, V]
        oh = np.ascontiguousarray(oh.transpose(0, 2, 1))  # [T, V, BL]
        m = dict(shared)
        m["onehot"] = oh
        in_maps.append(m)
    return in_maps, (fast_bias, fb_chunks)


def _run(in_maps, key, trace=False):
    from concourse.bass_utils import run_bass_kernel_spmd

    nc = _get_nc(key)
    res = run_bass_kernel_spmd(nc, in_maps, list(range(NCORES)), trace=trace)
    logits = np.concatenate([r["out"].T for r in res.results], axis=0)  # [B, V]
    return logits.astype(np.float32), res


def kernel(features, embedding, W1, b1, W2, b2, Wd, bd):
    in_maps, key = _prep_inputs(features, embedding, W1, b1, W2, b2, Wd, bd)
    logits, _ = _run(in_maps, key, trace=False)
    return logits
